# revision 6
# baseline (speedup 1.0000x reference)
"""AdMSoftmax loss on 8 TRN2 NeuronCores.

Strategy (vocab/tensor parallel, per the sharding hint):
  - Shard the class dim C=100000 into 8 shards of 12500.
  - Host-side staging: x is L2-normalized, scaled by 16 and cast to
    fp8-e4m3; each W shard is scaled by 16 and cast to fp8. Both are laid
    out in HBM already in the DoubleRow-interleaved [partition, e-chunk,
    col] order the PE wants, so each W group is ONE contiguous 2D DMA.
  - Per core: TensorE computes psum[n, c] = 256 * x_hat[n]*W[c] with fp8
    DoubleRow matmuls (2x rate).  The exp+row-sum of each psum group is
    split between two engines:
      * ScalarE: activation Exp (scale=S/256 compile-time const) with
        fused accum_out on ~62% of the columns,
      * VectorE: Schraudolph fast-exp in the bf16 domain on the rest --
        i16 = int16(psum*A16 + B16) (one tensor_scalar pass from PSUM),
        then the int bits reinterpreted as bf16 (~= exp) and row-summed
        with a second tensor_scalar (single-src 16-bit SBUF -> DVE 4x
        mode) with fused accum_out.  A16 = (S/256)*2^7/ln2, B16 centered
        so E[approx/true] ~= 1.
  - Per-(row-chunk, group, engine) partials land in `sums` [128, 8*16];
    the whole tile is DMA'd out and reduced on host (saves a device
    reduce on the critical tail).
  - Host combines the 8 cores' partials (the all-reduce of the
    denominator), adds the exact f64 label term, finishes the loss.

The pipeline is TensorE-bound (fp8 DR matmul of 6.55 GMAC/core ~= 95us);
ScalarE (~75us), VectorE (~60us) and DMA (6.9MB fp8, ~20us) hide under.
"""

import numpy as np
import ml_dtypes

N, E, C = 1024, 512, 100000
S, M = 30.0, 0.4
NCORES = 8
CS = C // NCORES            # 12500 classes per core

# class groups per core: small ramp groups cut startup latency (row 0
# only; rows 1..7 take them as one merged 1024 pass), then 2048-wide
# steady groups (4 PSUM banks), then a 1236 remainder interleaved in.
GROUPS = [(0, 256), (256, 256), (512, 512),
          (1024, 2048), (3072, 2048), (5120, 2048), (7168, 2048),
          (9216, 2048), (11264, 1236)]
assert sum(w for _, w in GROUPS) == CS
RAMP = GROUPS[0:3]
MAIN = GROUPS[3:8]
TAIL = GROUPS[8]

NSLOT = 16                  # accum columns per row-chunk in `sums`


def _split(w):
    """ScalarE columns (62.5%) of a group of width w; DVE takes the rest."""
    return (w * 5 // 8) & ~63 or w


# Schraudolph fast-exp constants, bf16-bits domain:
#   bits16 = int16(psum * A16 + B16); reinterpret as bf16 ~= exp(psum*S/256)
A16 = (S / 256.0) * (2.0 ** 7) / np.log(2.0)
B16 = 16256.0 - 0.056435 * 2.0 ** 7      # mean-centered linear error
ACT_SCALE = S / 256.0

_nc_cache = None


def _split_bir_waits(bir_json):
    """The walrus build in this image lowers at most ONE sync-wait per
    instruction (TPB_EVENTS has a single wait slot); Tile emits tail Drains
    with several. Split extra waits into single-wait EventSemaphore preludes
    on the same engine (sequential waits == AND of waits)."""
    import orjson
    j = orjson.loads(bir_json)
    changed = False
    for fn in j.get("functions", []):
        for bb in fn.get("blocks", []):
            out = []
            for inst in bb.get("instructions", []):
                si = inst.get("sync_info") or {}
                waits = si.get("on_wait") or []
                if len(waits) > 1:
                    changed = True
                    for k, w in enumerate(waits[:-1]):
                        out.append({
                            "debug": inst.get("debug", 0),
                            "engine": inst["engine"],
                            "ins": [], "outs": [],
                            "name": f'{inst["name"]}_wsplit{k}',
                            "opcode": "EventSemaphore",
                            "sync_info": {"on_update": [], "on_wait": [w]},
                        })
                    si["on_wait"] = [waits[-1]]
                    inst["sync_info"] = si
                out.append(inst)
            bb["instructions"] = out
    return orjson.dumps(j) if changed else bir_json


def _install_compile_patch():
    from concourse import bass2jax
    if getattr(bass2jax, "_wait_split_patched", False):
        return
    orig = bass2jax.compile_bir_kernel

    def patched(bir_json, tmpdir, neff_name="file.neff"):
        return orig(_split_bir_waits(bir_json), tmpdir, neff_name)

    bass2jax.compile_bir_kernel = patched
    bass2jax._wait_split_patched = True


def _build_nc():
    from concourse import bass, mybir, tile

    f32 = mybir.dt.float32
    bf16 = mybir.dt.bfloat16
    fp8 = mybir.dt.float8e4
    i16 = mybir.dt.int16
    AF = mybir.ActivationFunctionType
    ALU = mybir.AluOpType
    PM = mybir.MatmulPerfMode

    nc = bass.Bass(target_bir_lowering=False)
    x8_ext = nc.declare_dram_parameter("x8", [128, 4 * N], fp8, isOutput=False)
    w8_ext = nc.declare_dram_parameter("w8", [128, 4 * CS], fp8, isOutput=False)
    out_ext = nc.declare_dram_parameter("out", [128, 8 * NSLOT], f32,
                                        isOutput=True)

    with tile.TileContext(nc, pool_alloc_mode="queue") as tc:
        with tc.tile_pool(name="const", bufs=1) as cpool, \
             tc.tile_pool(name="ps", bufs=2, space="PSUM") as ppool, \
             tc.tile_pool(name="esc", bufs=3) as epool, \
             tc.tile_pool(name="i16", bufs=3) as ipool, \
             tc.tile_pool(name="p2o", bufs=3) as opool:

            # --- input DMAs. The ramp needs x + g0/g1/g2 earliest: those
            # issue on the sync HWDGE ring. The ACT ring first loads the
            # exp table (overlaps the sync DMAs), then issues the
            # steady-state W loads. All W tiles stay resident (6.4MB).
            xT_use = cpool.tile([128, 4 * N], fp8)
            wtiles = [cpool.tile([128, 4 * w], fp8, tag=f"wt{gi}",
                                 name=f"wt{gi}")
                      for gi, (c0, w) in enumerate(GROUPS)]

            nc.sync.dma_start(xT_use[:, 0:2 * N], x8_ext[:, 0:2 * N])
            nc.sync.dma_start(wtiles[0][:, :],
                              w8_ext[:, 4 * GROUPS[0][0]:
                                     4 * (GROUPS[0][0] + GROUPS[0][1])])
            nc.sync.dma_start(xT_use[:, 2 * N:4 * N], x8_ext[:, 2 * N:4 * N])
            for gi in (1, 2):
                c0, w = GROUPS[gi]
                nc.sync.dma_start(wtiles[gi][:, :],
                                  w8_ext[:, 4 * c0:4 * (c0 + w)])

            # exp activation table (~2.7us) loads while the ramp DMAs land
            warm = cpool.tile([128, 1], f32)
            nc.vector.memset(warm[:], 1.0)
            nc.scalar.activation(warm[:], warm[:], AF.Exp)

            # steady-state W loads on the ACT HWDGE ring (issue cost sits
            # on the otherwise-idle early act queue, not the sync ring)
            for gi in range(3, len(GROUPS)):
                c0, w = GROUPS[gi]
                nc.scalar.dma_start(wtiles[gi][:, :],
                                    w8_ext[:, 4 * c0:4 * (c0 + w)])

            # per-(row-chunk, slot) partial sums
            sums = cpool.tile([128, 8 * NSLOT], f32)
            nc.vector.memset(sums[:], 0.0)

            def dr_lhs(P, n):
                return xT_use[:, 2 * P * N:2 * (P + 1) * N] \
                    .rearrange("p (j q) -> p j q", j=2) \
                    [:, :, n * 128:(n + 1) * 128]

            def dr_rhs(wt, w, P):
                return wt[:, 2 * P * w:2 * (P + 1) * w] \
                    .rearrange("p (j c) -> p j c", j=2)

            def emit_mms(ps, n, chunks):
                """chunks: (wtile, w, col_off, width<=512) consecutive in ps."""
                for P in range(2):
                    o = 0
                    for (wt, w, coff, cw) in chunks:
                        nc.tensor.matmul(
                            ps[:, o:o + cw], dr_lhs(P, n),
                            dr_rhs(wt, w, P)[:, :, coff:coff + cw],
                            perf_mode=PM.DoubleRow,
                            start=(P == 0), stop=(P == 1))
                        o += cw

            def consume_split(ps, n, w, s_slot, d_slot):
                """ScalarE exp on cols [0:ws], DVE fast-exp on [ws:w]."""
                ws = _split(w)
                wd = w - ws
                escr = epool.tile([128, 1280], bf16, tag="escr")
                nc.scalar.activation(
                    escr[:, :ws], ps[:, :ws], AF.Exp, scale=ACT_SCALE,
                    accum_out=sums[:, n * NSLOT + s_slot:n * NSLOT + s_slot + 1])
                it = ipool.tile([128, 768], i16, tag="i16")
                nc.vector.tensor_scalar(it[:, :wd], ps[:, ws:w], A16, B16,
                                        op0=ALU.mult, op1=ALU.add)
                ot = opool.tile([128, 768], bf16, tag="p2o")
                nc.vector.tensor_scalar(
                    ot[:, :wd], it[:, :wd].bitcast(bf16), 1.0, 0.0,
                    op0=ALU.mult, op1=ALU.add,
                    accum_out=sums[:, n * NSLOT + d_slot:n * NSLOT + d_slot + 1])

            def consume_scalar(ps, n, w, slot):
                escr = epool.tile([128, 1280], bf16, tag="escr")
                nc.scalar.activation(
                    escr[:, :w], ps[:, :w], AF.Exp, scale=ACT_SCALE,
                    accum_out=sums[:, n * NSLOT + slot:n * NSLOT + slot + 1])

            def group_chunks(gi):
                c0, w = GROUPS[gi]
                return [(wtiles[gi], w, b * 512, min(w - b * 512, 512))
                        for b in range((w + 511) // 512)]

            # --- n=0 ramp: groups g0/g1/g2 individually, all-ScalarE
            for gi, (c0, w) in enumerate(RAMP):
                ps = ppool.tile([128, 2048], f32)
                emit_mms(ps, 0, group_chunks(gi))
                consume_scalar(ps, 0, w, gi)

            # --- n=1..7 merged pass over the ramp W (cols 0..1024)
            merged_chunks = (group_chunks(0) + group_chunks(1)
                             + group_chunks(2))
            for n in range(1, 8):
                ps = ppool.tile([128, 2048], f32)
                emit_mms(ps, n, merged_chunks)
                consume_split(ps, n, 1024, 0, 8)

            # --- steady state: 2048 groups with the 1236 tail interleaved
            sched = []
            tn = 0
            for mi in range(len(MAIN)):
                for n in range(8):
                    sched.append((3 + mi, n, 3 + mi, 9 + mi))
                    if (mi < 3 and n in (2, 6)) or (mi >= 3 and n == 4):
                        sched.append((8, tn, 14, 15))
                        tn += 1
            assert sorted(x[1] for x in sched if x[0] == 8) == list(range(8))
            for (gi, n, s_slot, d_slot) in sched:
                ps = ppool.tile([128, 2048], f32)
                emit_mms(ps, n, group_chunks(gi))
                consume_split(ps, n, GROUPS[gi][1], s_slot, d_slot)

            # --- ship all partials; host does the final reduce
            nc.sync.dma_start(out_ext[:, :], sums[:, :])

    return nc


def _host_prep(x, W):
    """Normalize+scale+cast to fp8 and lay out in the device DMA order:
    [partition p, e-chunk ej, col] flattened, with W additionally
    group-blocked so each group is one contiguous 2D slice."""
    fp8 = ml_dtypes.float8_e4m3
    xn = x / np.linalg.norm(x, axis=1, keepdims=True)
    x8 = (xn.T * 16.0).astype(fp8)                    # [E, N]
    x8 = np.ascontiguousarray(
        x8.reshape(4, 128, N).transpose(1, 0, 2).reshape(128, 4 * N))

    w8s = []
    for i in range(NCORES):
        wi = (W[i * CS:(i + 1) * CS].T * 16.0).astype(fp8)   # [E, CS]
        wi = wi.reshape(4, 128, CS).transpose(1, 0, 2)       # [128, 4, CS]
        blocks = [np.ascontiguousarray(wi[:, :, c0:c0 + w]).reshape(128, 4 * w)
                  for (c0, w) in GROUPS]
        w8s.append(np.ascontiguousarray(np.concatenate(blocks, axis=1)))
    return x8, w8s


TRACE = False
TRACE_KW = {}
LAST_RESULT = None


def kernel(x, labels, W):
    global _nc_cache, LAST_RESULT
    x = np.ascontiguousarray(np.asarray(x, dtype=np.float32))
    W = np.ascontiguousarray(np.asarray(W, dtype=np.float32))
    labels_i = np.asarray(labels).astype(np.int64)

    _install_compile_patch()
    if _nc_cache is None:
        _nc_cache = _build_nc()
    nc = _nc_cache

    x8, w8s = _host_prep(x, W)
    in_maps = [{"x8": x8, "w8": w8s[i]} for i in range(NCORES)]

    from concourse.bass_utils import run_bass_kernel_spmd
    res = run_bass_kernel_spmd(nc, in_maps, core_ids=list(range(NCORES)),
                               trace=TRACE, **TRACE_KW)
    LAST_RESULT = res

    total = np.zeros(N, dtype=np.float64)
    for i in range(NCORES):
        o = np.asarray(res.results[i]["out"], dtype=np.float64)  # [128, 8*16]
        total += o.reshape(128, 8, NSLOT).sum(axis=2).T.reshape(N)
    sum_all = total

    # Exact label term + final scalar combine (the gather/unshard step).
    xn = x.astype(np.float64)
    xn /= np.linalg.norm(xn, axis=1, keepdims=True)
    wf_y = np.sum(xn * W[labels_i].astype(np.float64), axis=1)
    numerator = S * (wf_y - M)
    denominator = np.exp(numerator) + sum_all - np.exp(S * wf_y)
    L = numerator - np.log(denominator)
    return np.float32(-np.mean(L))


# revision 7
# speedup vs baseline: 1.0880x; 1.0880x over previous
"""AdMSoftmax loss on 8 TRN2 NeuronCores.

Strategy (vocab/tensor parallel, per the sharding hint):
  - Shard the class dim C=100000 into 8 shards of 12500.
  - Host-side staging: x is L2-normalized, scaled by 16 and cast to
    fp8-e4m3; each W shard is scaled by 16 and cast to fp8. Both are laid
    out in HBM already in the DoubleRow-interleaved [partition, e-chunk,
    col] order the PE wants, so each W group is ONE contiguous 2D DMA on
    the sync HWDGE ring, issued in consumption order.
  - Per core: TensorE computes psum[n, c] = 256 * x_hat[n]*W[c] with fp8
    DoubleRow matmuls (2x rate).  The exp+row-sum of each psum group is
    split across THREE engines:
      * ScalarE (~56%): activation Exp (scale=S/256 const) + fused
        accum_out;
      * VectorE: Schraudolph fast-exp in the bf16-bits domain on the
        rest -- p1: i16 = int16(psum*A16 + B16) (one pass from PSUM);
        the bits reinterpreted as bf16 are ~= exp(psum*S/256).
        p2: DVE sums a slice of the bits with tensor_scalar accum_out;
      * GpSimd (idle otherwise): accumulates the remaining bits slice
        into a per-row-chunk f32 accumulator (tensor_tensor add);
        VectorE reduces each accumulator once at the very end.
  - Per-(row-chunk, slot) partials land in `sums` [128, 8*17]; the tile
    is DMA'd out in two chunks (first half early to hide completion
    latency) and reduced on host.
  - Host combines the 8 cores' partials (the all-reduce of the
    denominator), adds the exact f64 label term, finishes the loss.

The pipeline is TensorE-bound (fp8 DR matmul of 6.55 GMAC/core ~= 96us
busy); ScalarE (~82us), VectorE (~84us), GpSimd (~52us) and DMA (6.9MB
fp8, ~20us) hide underneath.
"""

import numpy as np
import ml_dtypes

N, E, C = 1024, 512, 100000
S, M = 30.0, 0.4
NCORES = 8
CS = C // NCORES            # 12500 classes per core

# class groups per core: small ramp groups cut startup latency (row 0
# only; rows 1..7 take them as one merged 1024 pass), then 2048-wide
# steady groups (4 PSUM banks), with the 1236 remainder interleaved.
GROUPS = [(0, 256), (256, 256), (512, 512),
          (1024, 2048), (3072, 2048), (5120, 2048), (7168, 2048),
          (9216, 2048), (11264, 1236)]
assert sum(w for _, w in GROUPS) == CS
RAMP = GROUPS[0:3]
MAIN = GROUPS[3:8]

NSLOT = 17                  # accum columns per row-chunk in `sums`

# (scalar cols, dve-p2 cols, gpsimd cols) per split group width
SPLITS = {2048: (1152, 256, 640), 1024: (576, 128, 320),
          1236: (704, 152, 380)}
GPW = 640                   # gpsimd accumulator width (max gp cols)

# Schraudolph fast-exp constants, bf16-bits domain:
#   bits16 = int16(psum * A16 + B16); reinterpret as bf16 ~= exp(psum*S/256)
A16 = (S / 256.0) * (2.0 ** 7) / np.log(2.0)
B16 = 16256.0 - 0.056435 * 2.0 ** 7      # mean-centered linear error
ACT_SCALE = S / 256.0

_nc_cache = None


def _split_bir_waits(bir_json):
    """The walrus build in this image lowers at most ONE sync-wait per
    instruction (TPB_EVENTS has a single wait slot); Tile emits tail Drains
    with several. Split extra waits into single-wait EventSemaphore preludes
    on the same engine (sequential waits == AND of waits)."""
    import orjson
    j = orjson.loads(bir_json)
    changed = False
    for fn in j.get("functions", []):
        for bb in fn.get("blocks", []):
            out = []
            for inst in bb.get("instructions", []):
                si = inst.get("sync_info") or {}
                waits = si.get("on_wait") or []
                if len(waits) > 1:
                    changed = True
                    for k, w in enumerate(waits[:-1]):
                        out.append({
                            "debug": inst.get("debug", 0),
                            "engine": inst["engine"],
                            "ins": [], "outs": [],
                            "name": f'{inst["name"]}_wsplit{k}',
                            "opcode": "EventSemaphore",
                            "sync_info": {"on_update": [], "on_wait": [w]},
                        })
                    si["on_wait"] = [waits[-1]]
                    inst["sync_info"] = si
                out.append(inst)
            bb["instructions"] = out
    return orjson.dumps(j) if changed else bir_json


def _install_compile_patch():
    from concourse import bass2jax
    if getattr(bass2jax, "_wait_split_patched", False):
        return
    orig = bass2jax.compile_bir_kernel

    def patched(bir_json, tmpdir, neff_name="file.neff"):
        return orig(_split_bir_waits(bir_json), tmpdir, neff_name)

    bass2jax.compile_bir_kernel = patched
    bass2jax._wait_split_patched = True


def _build_nc():
    from concourse import bass, mybir, tile

    f32 = mybir.dt.float32
    bf16 = mybir.dt.bfloat16
    fp8 = mybir.dt.float8e4
    i16 = mybir.dt.int16
    AF = mybir.ActivationFunctionType
    ALU = mybir.AluOpType
    AX = mybir.AxisListType
    PM = mybir.MatmulPerfMode

    nc = bass.Bass(target_bir_lowering=False)
    x8_ext = nc.declare_dram_parameter("x8", [128, 4 * N], fp8, isOutput=False)
    w8_ext = nc.declare_dram_parameter("w8", [128, 4 * CS], fp8, isOutput=False)
    out_ext = nc.declare_dram_parameter("out", [128, 8 * NSLOT], f32,
                                        isOutput=True)

    with tile.TileContext(nc, pool_alloc_mode="queue") as tc:
        with tc.tile_pool(name="const", bufs=1) as cpool, \
             tc.tile_pool(name="ps", bufs=2, space="PSUM") as ppool, \
             tc.tile_pool(name="esc", bufs=3) as epool, \
             tc.tile_pool(name="i16", bufs=3) as ipool, \
             tc.tile_pool(name="p2o", bufs=3) as opool:

            # --- input DMAs on the sync HWDGE ring (FIFO per ring), in
            # consumption order: x halves + ramp W first, then the
            # steady-state W.  All W tiles stay resident (6.4MB << SBUF).
            xT_use = cpool.tile([128, 4 * N], fp8)
            wtiles = [cpool.tile([128, 4 * w], fp8, tag=f"wt{gi}",
                                 name=f"wt{gi}")
                      for gi, (c0, w) in enumerate(GROUPS)]

            def w_dma(gi):
                c0, w = GROUPS[gi]
                nc.sync.dma_start(wtiles[gi][:, :],
                                  w8_ext[:, 4 * c0:4 * (c0 + w)])

            nc.sync.dma_start(xT_use[:, 0:2 * N], x8_ext[:, 0:2 * N])
            w_dma(0)
            nc.sync.dma_start(xT_use[:, 2 * N:4 * N], x8_ext[:, 2 * N:4 * N])
            for gi in range(1, len(GROUPS)):
                w_dma(gi)

            # exp activation table (~2.7us) loads while the DMAs land
            warm = cpool.tile([128, 1], f32)
            nc.vector.memset(warm[:], 1.0)
            nc.scalar.activation(warm[:], warm[:], AF.Exp)

            # per-(row-chunk, slot) partial sums + gpsimd accumulators
            sums = cpool.tile([128, 8 * NSLOT], f32)
            nc.vector.memset(sums[:], 0.0)
            accs = [cpool.tile([128, GPW], f32, tag=f"acc{n}", name=f"acc{n}")
                    for n in range(8)]
            for n in range(8):
                nc.gpsimd.memset(accs[n][:, :], 0.0)

            def dr_lhs(P, n):
                return xT_use[:, 2 * P * N:2 * (P + 1) * N] \
                    .rearrange("p (j q) -> p j q", j=2) \
                    [:, :, n * 128:(n + 1) * 128]

            def dr_rhs(wt, w, P):
                return wt[:, 2 * P * w:2 * (P + 1) * w] \
                    .rearrange("p (j c) -> p j c", j=2)

            def emit_mms(ps, n, chunks):
                """chunks: (wtile, w, col_off, width<=512) consecutive in ps."""
                for P in range(2):
                    o = 0
                    for (wt, w, coff, cw) in chunks:
                        nc.tensor.matmul(
                            ps[:, o:o + cw], dr_lhs(P, n),
                            dr_rhs(wt, w, P)[:, :, coff:coff + cw],
                            perf_mode=PM.DoubleRow,
                            start=(P == 0), stop=(P == 1))
                        o += cw

            def consume_split(ps, n, w, s_slot, d_slot):
                """ScalarE exp on [0:ws]; DVE fast-exp bits on [ws:w]; the
                bits split between a DVE accum-sum and a gpsimd acc add."""
                ws, wp2, wgp = SPLITS[w]
                wd = w - ws
                escr = epool.tile([128, 1152], bf16, tag="escr")
                nc.scalar.activation(
                    escr[:, :ws], ps[:, :ws], AF.Exp, scale=ACT_SCALE,
                    accum_out=sums[:, n * NSLOT + s_slot:n * NSLOT + s_slot + 1])
                it = ipool.tile([128, 896], i16, tag="i16")
                nc.vector.tensor_scalar(it[:, :wd], ps[:, ws:w], A16, B16,
                                        op0=ALU.mult, op1=ALU.add)
                ot = opool.tile([128, 256], bf16, tag="p2o")
                nc.vector.tensor_scalar(
                    ot[:, :wp2], it[:, :wp2].bitcast(bf16), 1.0, 0.0,
                    op0=ALU.mult, op1=ALU.add,
                    accum_out=sums[:, n * NSLOT + d_slot:n * NSLOT + d_slot + 1])
                nc.gpsimd.tensor_tensor(
                    accs[n][:, :wgp], accs[n][:, :wgp],
                    it[:, wp2:wp2 + wgp].bitcast(bf16), op=ALU.add)

            def consume_scalar(ps, n, w, slot):
                escr = epool.tile([128, 1152], bf16, tag="escr")
                nc.scalar.activation(
                    escr[:, :w], ps[:, :w], AF.Exp, scale=ACT_SCALE,
                    accum_out=sums[:, n * NSLOT + slot:n * NSLOT + slot + 1])

            def group_chunks(gi):
                c0, w = GROUPS[gi]
                return [(wtiles[gi], w, b * 512, min(w - b * 512, 512))
                        for b in range((w + 511) // 512)]

            # --- n=0 ramp: groups g0/g1/g2 individually, all-ScalarE
            for gi, (c0, w) in enumerate(RAMP):
                ps = ppool.tile([128, 2048], f32)
                emit_mms(ps, 0, group_chunks(gi))
                consume_scalar(ps, 0, w, gi)

            # --- n=1..7 merged pass over the ramp W (cols 0..1024)
            merged_chunks = (group_chunks(0) + group_chunks(1)
                             + group_chunks(2))
            for n in range(1, 8):
                ps = ppool.tile([128, 2048], f32)
                emit_mms(ps, n, merged_chunks)
                consume_split(ps, n, 1024, 0, 8)

            # --- steady state: 2048 groups with the 1236 tail interleaved
            sched = []
            tn = 0
            for mi in range(len(MAIN)):
                for n in range(8):
                    sched.append((3 + mi, n, 3 + mi, 9 + mi))
                    if (mi < 3 and n in (2, 6)) or (mi >= 3 and n == 4):
                        sched.append((8, tn, 14, 15))
                        tn += 1
            assert sorted(x[1] for x in sched if x[0] == 8) == list(range(8))

            for (gi, n, s_slot, d_slot) in sched:
                ps = ppool.tile([128, 2048], f32)
                emit_mms(ps, n, group_chunks(gi))
                consume_split(ps, n, GROUPS[gi][1], s_slot, d_slot)
                if gi == 7:
                    # n's accumulator is complete: fold it into slot 16
                    nc.vector.tensor_reduce(
                        sums[:, n * NSLOT + 16:n * NSLOT + 17],
                        accs[n][:, :], axis=AX.X, op=ALU.add)
                    if n == 3:
                        # rows 0..3 fully done: ship their partials early
                        nc.sync.dma_start(out_ext[:, 0:4 * NSLOT],
                                          sums[:, 0:4 * NSLOT])

            nc.sync.dma_start(out_ext[:, 4 * NSLOT:8 * NSLOT],
                              sums[:, 4 * NSLOT:8 * NSLOT])

    return nc


def _host_prep(x, W):
    """Normalize+scale+cast to fp8 and lay out in the device DMA order:
    [partition p, e-chunk ej, col] flattened, with W additionally
    group-blocked so each group is one contiguous 2D slice."""
    fp8 = ml_dtypes.float8_e4m3
    xn = x / np.linalg.norm(x, axis=1, keepdims=True)
    x8 = (xn.T * 16.0).astype(fp8)                    # [E, N]
    x8 = np.ascontiguousarray(
        x8.reshape(4, 128, N).transpose(1, 0, 2).reshape(128, 4 * N))

    w8s = []
    for i in range(NCORES):
        wi = (W[i * CS:(i + 1) * CS].T * 16.0).astype(fp8)   # [E, CS]
        wi = wi.reshape(4, 128, CS).transpose(1, 0, 2)       # [128, 4, CS]
        blocks = [np.ascontiguousarray(wi[:, :, c0:c0 + w]).reshape(128, 4 * w)
                  for (c0, w) in GROUPS]
        w8s.append(np.ascontiguousarray(np.concatenate(blocks, axis=1)))
    return x8, w8s


TRACE = False
TRACE_KW = {}
LAST_RESULT = None


def kernel(x, labels, W):
    global _nc_cache, LAST_RESULT
    x = np.ascontiguousarray(np.asarray(x, dtype=np.float32))
    W = np.ascontiguousarray(np.asarray(W, dtype=np.float32))
    labels_i = np.asarray(labels).astype(np.int64)

    _install_compile_patch()
    if _nc_cache is None:
        _nc_cache = _build_nc()
    nc = _nc_cache

    x8, w8s = _host_prep(x, W)
    in_maps = [{"x8": x8, "w8": w8s[i]} for i in range(NCORES)]

    from concourse.bass_utils import run_bass_kernel_spmd
    res = run_bass_kernel_spmd(nc, in_maps, core_ids=list(range(NCORES)),
                               trace=TRACE, **TRACE_KW)
    LAST_RESULT = res

    total = np.zeros(N, dtype=np.float64)
    for i in range(NCORES):
        o = np.asarray(res.results[i]["out"], dtype=np.float64)  # [128, 8*17]
        total += o.reshape(128, 8, NSLOT).sum(axis=2).T.reshape(N)
    sum_all = total

    # Exact label term + final scalar combine (the gather/unshard step).
    xn = x.astype(np.float64)
    xn /= np.linalg.norm(xn, axis=1, keepdims=True)
    wf_y = np.sum(xn * W[labels_i].astype(np.float64), axis=1)
    numerator = S * (wf_y - M)
    denominator = np.exp(numerator) + sum_all - np.exp(S * wf_y)
    L = numerator - np.log(denominator)
    return np.float32(-np.mean(L))
